# revision 26
# baseline (speedup 1.0000x reference)
"""Causal self-attention (B=4, T=2048, C=1024, H=16) on 8 TRN2 NeuronCores.

Sharding: tensor-parallel over heads. Each core owns 2 heads:
  - c_attn: output columns (q,k,v dims) for its heads  -> [384, 1024] shard
  - attention: embarrassingly parallel over (B, local heads)
  - c_proj: input rows for its heads -> partial [B,T,C] output, summed on host

Device layouts (host pre-transposed so every matmul contraction dim sits on
SBUF partitions):
  xt   [B, C, T]       x transposed; QKV matmul rhs tiles  [128 k, 512 tok]
  wqkv [128, 8, 384]   wqkv[p,k,n] = W_shard.T[k*128+p, n] (lhsT tiles)
  bqkv [128, 3]        per-partition bias, column n_t
  wp   [128, 1024]     wp[p,c] = W_proj[c, core*128+p]     (proj rhs)

Per-core structure: three instruction streams per batch, emitted
INTERLEAVED so the strict-FIFO PE queue always holds ready filler work
behind any dependency-stalled attention matmul:
  - attention(b): per i-superblock, per causal 128-row j tile:
      S^T pair [128, 2, w] PSUM: the two heads' K=64 matmuls sit in
        disjoint PE row groups -> run CONCURRENTLY
      P^T = exp(S^T/64)  (ONE ACT instr over the 2-bank 3D AP)
      causal diag: one DVE multiply zeroes both heads' triangles
      per head: Y^T[65, 512] += V2aug_j.T @ P^T (row 64 = denominator;
        PV emission lags one j step so PE never waits on ACT)
      normalize: y = Y^T[0:64] * bcast(1/Y^T[64])
  - QKV(b+1): W.T @ x.T + bias (DVE), then V^T transposes into V2aug
  - proj(b-1): y.T @ Wp^T -> staged bf16 -> DMA out [B, T, C] bf16
Host: out = sum(partials f32) + b_proj.
"""

import os
import sys
from itertools import chain

import numpy as np

os.environ.setdefault("MYCRO_LOCAL_CACHE", "1")
if "/opt/trn_rl_repo" not in sys.path:
    sys.path.insert(0, "/opt/trn_rl_repo")

B, T, C = 4, 2048, 1024
H, D = 16, 64
N_CORES = 8
HPC = H // N_CORES          # heads per core = 2
NL = HPC * D                # local width per q/k/v = 128
KT = C // 128               # 8 contraction tiles for QKV
NT = 3                      # q, k, v
SW = 512                    # i superblock width
NSB = T // SW               # 4 superblocks per batch
NJT = T // 128              # 16 j tiles per batch
FILL = 1                    # filler units pulled per attention j-slot

# matmul input dtype: bf16 (fastest), f32r (tf32-like), f32 (exact, 4x slow)
KDT = os.environ.get("KERNEL_DTYPE", "bf16")

_cache = {}
LAST_RESULT = None


def _np_mdt():
    if KDT == "bf16":
        import ml_dtypes
        return np.dtype(ml_dtypes.bfloat16)
    return np.dtype(np.float32)


def _build():
    import concourse.tile as tile
    from concourse import bacc, mybir

    dt = mybir.dt
    f32 = dt.float32
    mdt = {"bf16": dt.bfloat16, "f32r": dt.float32r, "f32": f32}[KDT]

    nc = bacc.Bacc("TRN2", target_bir_lowering=False, debug=False,
                   num_devices=N_CORES)

    xt = nc.dram_tensor("xt", [B, C, T], mdt, kind="ExternalInput").ap()
    wqkv = nc.dram_tensor("wqkv", [128, KT, NT * 128], mdt,
                          kind="ExternalInput").ap()
    bqkv = nc.dram_tensor("bqkv", [128, NT], f32, kind="ExternalInput").ap()
    wp = nc.dram_tensor("wp", [128, C], mdt, kind="ExternalInput").ap()
    out = nc.dram_tensor("out", [B, T, C], mdt, kind="ExternalOutput").ap()

    np_m = _np_mdt() if KDT == "bf16" else np.float32
    ident_np = np.eye(128).astype(np_m)
    # P^T layout: rows x = j (keys), cols y = i (queries); keep j <= i
    # 0/1 multiplicative causal mask applied post-exp, duplicated per head
    trit01_np = np.where(
        np.arange(128)[:, None] <= np.arange(128)[None, :],
        np.float32(1.0), np.float32(0.0)).astype(np_m)
    trit01_2_np = np.ascontiguousarray(
        np.stack([trit01_np, trit01_np], axis=1))  # [128, 2, 128]
    ones_np = np.ones((128, NJT, HPC, 1)).astype(np_m)
    ident_dram = nc.inline_tensor(ident_np, name="ident").ap()
    trit2_dram = nc.inline_tensor(trit01_2_np, name="tritmask2").ap()
    ones_dram = nc.inline_tensor(ones_np, name="onescol").ap()

    Exp = mybir.ActivationFunctionType.Exp

    _alt = [0]

    def copy_alt(dst, src):
        _alt[0] ^= 1
        if _alt[0]:
            nc.scalar.copy(dst, src)
        else:
            nc.vector.tensor_copy(dst, src)

    with tile.TileContext(nc) as tc:
        with (
            tc.tile_pool(name="consts", bufs=1) as consts,
            tc.tile_pool(name="xtp", bufs=2) as xtp,
            tc.tile_pool(name="qkvtp", bufs=2) as qkvtp,
            tc.tile_pool(name="yp", bufs=2) as yp,
            tc.tile_pool(name="v2p", bufs=2) as v2p,
            tc.tile_pool(name="ptp", bufs=8) as ptp,
            tc.tile_pool(name="stage", bufs=8) as stage,
            tc.tile_pool(name="stats", bufs=6) as stats,
            tc.tile_pool(name="rbp", bufs=4) as rbp,
            tc.tile_pool(name="ycp", bufs=2) as ycp,
            tc.tile_pool(name="s_ps", bufs=2, space="PSUM") as s_ps,
            tc.tile_pool(name="yt_ps", bufs=2, space="PSUM") as yt_ps,
            tc.tile_pool(name="qk_ps", bufs=2, space="PSUM") as qk_ps,
        ):
            # HAM warm-up primer: dense dummy matmuls with no input deps so
            # the PE clock is at 2.4GHz by the time real work arrives.
            prime = consts.tile([128, SW], mdt if KDT != "f32r" else f32)
            nc.gpsimd.memset(prime[:], 0.25)
            for _ in range(0 if KDT == "f32r" else 40):
                pps = qk_ps.tile([128, SW], f32, tag="m")
                nc.tensor.matmul(pps[:], lhsT=prime[:, 0:128], rhs=prime[:],
                                 start=True, stop=True)

            nc.scalar.activation(prime[0:1, 0:1], prime[0:1, 0:1], Exp,
                                 scale=1.0)

            wqkv_sb = consts.tile([128, KT, NT * 128], mdt)
            nc.sync.dma_start(wqkv_sb[:], wqkv[:])
            wp_sb = consts.tile([128, C], mdt)
            nc.sync.dma_start(wp_sb[:], wp[:])
            bias_sb = consts.tile([128, NT], f32)
            nc.sync.dma_start(bias_sb[:], bqkv[:])
            ident_sb = consts.tile([128, 128], mdt)
            nc.sync.dma_start(ident_sb[:], ident_dram[:].bitcast(mdt))
            trit2_sb = consts.tile([128, 2, 128], mdt)
            nc.sync.dma_start(trit2_sb[:], trit2_dram[:].bitcast(mdt))

            qkvt_of = {}
            v2a_of = {}
            y_of = {}

            def qkv_stream(b):
                """QKV matmuls + bias, then V^T transposes. Yields after
                every couple of PE instructions."""
                xt_sb = xtp.tile([128, KT, T], mdt, tag="xt")
                for k in range(KT):
                    nc.sync.dma_start(xt_sb[:, k, :],
                                      xt[b, k * 128:(k + 1) * 128, :])
                qkvt = qkvt_of[b] = qkvtp.tile([128, NT, T], mdt, tag="qkvt", name="qkvt")
                for n_t in range(NT):
                    for ts in range(T // SW):
                        ps = qk_ps.tile([128, SW], f32, tag="m")
                        for k in range(KT):
                            nc.tensor.matmul(
                                ps[:],
                                lhsT=wqkv_sb[:, k, n_t * 128:(n_t + 1) * 128],
                                rhs=xt_sb[:, k, ts * SW:(ts + 1) * SW],
                                start=(k == 0), stop=(k == KT - 1),
                            )
                            if k % 2 == 1:
                                yield
                        nc.vector.tensor_scalar_add(
                            qkvt[:, n_t, ts * SW:(ts + 1) * SW], ps[:],
                            bias_sb[:, n_t:n_t + 1])
                # V2aug: V^T transposed + ones column
                v2a = v2a_of[b] = v2p.tile([128, NJT, HPC, 65], mdt,
                                           tag="v2a", name="v2a")
                nc.sync.dma_start(v2a[:, :, :, 64:65],
                                   ones_dram[:].bitcast(mdt))
                for j_t in range(NJT):
                    trp = qk_ps.tile([128, 128], mdt, tag="m")
                    nc.tensor.transpose(
                        trp[:], qkvt[:, 2, j_t * 128:(j_t + 1) * 128],
                        ident_sb[:])
                    # both head halves in one strided copy [128, 2, 64]
                    nc.vector.tensor_copy(v2a[:, j_t, :, 0:64], trp[:])
                    yield

            def attention_stream(b):
                """S/exp/PV per causal j tile; heads run as concurrent PE
                row-tile pairs; PV emission lags one j step."""
                qkvt = qkvt_of[b]
                v2a = v2a_of[b]
                y_sb = y_of[b] = yp.tile([128, T], mdt, tag="y", name="y_sb")
                q0 = qkvt[0:64, 0, :]
                k0 = qkvt[0:64, 1, :]
                q1 = qkvt[64:128, 0, :]
                k1 = qkvt[64:128, 1, :]
                def _pv(yt, njt, j_t, w, pt):
                    for h in range(HPC):
                        nc.tensor.matmul(
                            yt[h][:, SW - w:SW],
                            lhsT=v2a[:, j_t, h, :],
                            rhs=pt[:, h, :w],
                            start=(j_t == 0), stop=(j_t == njt - 1),
                        )

                def _norm(yt, i_sb):
                    # normalize: y = yc[0:64] * bcast(1/yc[64]).
                    # First op copies the whole accumulator to SBUF so the
                    # yt PSUM bank frees after ONE instruction instead of
                    # holding through the serial recip/broadcast chain
                    # (which starved the next superblock's PV matmuls).
                    # (denom row bounced to base-0: the approx-recip custom
                    # op misreads PSUM/base-64 inputs on HW)
                    yc = [None, None]
                    dnr = [None, None]
                    rcp = [None, None]
                    rb = [None, None]
                    for h in range(HPC):
                        yc[h] = ycp.tile([65, SW], mdt, tag=f"yc{h}",
                                         name="yc")
                        nc.vector.tensor_copy(yc[h][:], yt[h][:])
                    for h in range(HPC):
                        dnr[h] = stats.tile([1, SW], f32, tag=f"dnr{h}",
                                            name="dnr")
                        nc.vector.tensor_copy(dnr[h][:], yc[h][64:65, :])
                    for h in range(HPC):
                        rcp[h] = stats.tile([1, SW], f32, tag=f"rcp{h}",
                                            name="rcp")
                        nc.vector.reciprocal_approx_fast(out=rcp[h][:],
                                                         in_=dnr[h][:])
                    for h in range(HPC):
                        rb[h] = rbp.tile([64, SW], f32, tag=f"rb{h}",
                                         name="rb")
                        nc.gpsimd.partition_broadcast(rb[h][:], rcp[h][:])
                    for h in range(HPC):
                        nc.vector.tensor_mul(
                            y_sb[h * 64:(h + 1) * 64,
                                 i_sb * SW:(i_sb + 1) * SW],
                            yc[h][0:64, :], rb[h][:])

                def _proj(b, i_sb):
                    # this superblock's tokens are normalized -> project
                    for m_t in range(4 * i_sb, 4 * (i_sb + 1)):
                        for c_h in range(C // SW):
                            op = qk_ps.tile([128, SW], f32, tag="m",
                                            name="op")
                            nc.tensor.matmul(
                                op[:],
                                lhsT=y_sb[:, m_t * 128:(m_t + 1) * 128],
                                rhs=wp_sb[:, c_h * SW:(c_h + 1) * SW],
                                start=True, stop=True,
                            )
                            ost = stage.tile([128, SW], mdt, tag="ost")
                            copy_alt(ost[:], op[:])
                            nc.sync.dma_start(
                                out[b, m_t * 128:(m_t + 1) * 128,
                                    c_h * SW:(c_h + 1) * SW], ost[:])

                pending = []  # deferred PV/norm emission, lag 2 j-slots
                LAG = 5
                # reversed superblock order: the big superblock's proj
                # overlaps the small ones' attention, and each batch ends
                # on the lightest norm chain (smaller tail bubble)
                for i_sb in reversed(range(NSB)):
                    yt = [yt_ps.tile([65, SW], f32, tag="yt",
                                     name=f"yt{h}")
                          for h in range(HPC)]
                    njt = 4 * (i_sb + 1)

                    for j_t in range(njt):
                        if len(pending) > LAG:
                            pending.pop(0)()
                        yield
                        jtl = j_t - 4 * i_sb   # >=0 on the diagonal
                        diag = jtl >= 0
                        w = SW - jtl * 128 if diag else SW
                        i_lo = j_t * 128 if diag else i_sb * SW
                        sp = s_ps.tile([128, 2, SW], f32, tag="s")
                        # two K=64 matmuls in disjoint PE row groups ->
                        # concurrent (tile_position auto (0,0) / (64,0))
                        nc.tensor.matmul(
                            sp[:, 0, :w],
                            lhsT=k0[:, j_t * 128:(j_t + 1) * 128],
                            rhs=q0[:, i_lo:i_lo + w],
                            start=True, stop=True,
                        )
                        nc.tensor.matmul(
                            sp[:, 1, :w],
                            lhsT=k1[:, j_t * 128:(j_t + 1) * 128],
                            rhs=q1[:, i_lo:i_lo + w],
                            start=True, stop=True,
                        )
                        # one exp over both heads' banks (3D AP)
                        pt = ptp.tile([128, 2, SW], mdt, tag="pt")
                        nc.scalar.activation(
                            pt[:, :, :w], sp[:, :, :w], Exp, scale=1.0 / D)
                        if diag:
                            # causal mask: zero both heads' triangles with
                            # one DVE multiply (post-exp, 0/1 mask)
                            nc.vector.tensor_mul(
                                pt[:, :, 0:128], pt[:, :, 0:128],
                                trit2_sb[:])
                        pending.append(
                            lambda yt=yt, njt=njt, j_t=j_t, w=w, pt=pt:
                                _pv(yt, njt, j_t, w, pt))
                    pending.append(
                        lambda yt=yt, i_sb=i_sb: _norm(yt, i_sb))
                    pending.append(
                        lambda i_sb=i_sb: _proj(b, i_sb))
                while pending:
                    pending.pop(0)()
                    yield

            # prologue: batch 0's QKV runs alone
            for _ in qkv_stream(0):
                pass
            for b in range(B):
                filler = qkv_stream(b + 1) if b + 1 < B else iter(())
                for _ in attention_stream(b):
                    for _ in range(FILL):
                        if next(filler, None) is None:
                            break
                for _ in filler:
                    pass

    nc.compile()
    return nc


def _get_nc():
    if "nc" not in _cache:
        _cache["nc"] = _build()
    return _cache["nc"]


def kernel(x, W_attn, b_attn, W_proj, b_proj):
    global LAST_RESULT
    from concourse.bass_utils import run_bass_kernel_spmd

    x = np.asarray(x, dtype=np.float32)
    W_attn = np.asarray(W_attn, dtype=np.float32)
    b_attn = np.asarray(b_attn, dtype=np.float32)
    W_proj = np.asarray(W_proj, dtype=np.float32)
    b_proj = np.asarray(b_proj, dtype=np.float32)

    nc = _get_nc()
    np_m = _np_mdt()

    xt = np.ascontiguousarray(x.transpose(0, 2, 1)).astype(np_m)
    in_maps = []
    for c in range(N_CORES):
        sl = slice(c * NL, (c + 1) * NL)
        w_shard = np.concatenate(
            [W_attn[sl], W_attn[C:2 * C][sl], W_attn[2 * C:][sl]], axis=0)
        # wqkv[p, k, n] = w_shard.T[k*128+p, n]
        wqkv = np.ascontiguousarray(
            w_shard.T.reshape(KT, 128, NT * 128).transpose(1, 0, 2)).astype(np_m)
        b_shard = np.concatenate(
            [b_attn[sl], b_attn[C:2 * C][sl], b_attn[2 * C:][sl]])
        bq = np.ascontiguousarray(b_shard.reshape(NT, 128).T)
        wp_c = np.ascontiguousarray(W_proj[:, sl].T).astype(np_m)
        in_maps.append({"xt": xt, "wqkv": wqkv, "bqkv": bq, "wp": wp_c})

    try:
        res = run_bass_kernel_spmd(nc, in_maps,
                                   core_ids=list(range(N_CORES)))
    except Exception:
        # one retry: transient NRT/device hiccups recover on re-run
        import time
        time.sleep(10)
        res = run_bass_kernel_spmd(nc, in_maps,
                                   core_ids=list(range(N_CORES)))
    LAST_RESULT = res

    acc = res.results[0]["out"].astype(np.float32)
    for c in range(1, N_CORES):
        acc = acc + res.results[c]["out"].astype(np.float32)
    return acc + b_proj


# revision 28
# speedup vs baseline: 1.0085x; 1.0085x over previous
"""Causal self-attention (B=4, T=2048, C=1024, H=16) on 8 TRN2 NeuronCores.

Sharding: tensor-parallel over heads. Each core owns 2 heads:
  - c_attn: output columns (q,k,v dims) for its heads  -> [384, 1024] shard
  - attention: embarrassingly parallel over (B, local heads)
  - c_proj: input rows for its heads -> partial [B,T,C] output, summed on host

Device layouts (host pre-transposed so every matmul contraction dim sits on
SBUF partitions):
  xt   [B, C, T]       x transposed; QKV matmul rhs tiles  [128 k, 512 tok]
  wqkv [128, 8, 384]   wqkv[p,k,n] = W_shard.T[k*128+p, n] (lhsT tiles)
  bqkv [128, 3]        per-partition bias, column n_t
  wp   [128, 1024]     wp[p,c] = W_proj[c, core*128+p]     (proj rhs)

Per-core structure: three instruction streams per batch, emitted
INTERLEAVED so the strict-FIFO PE queue always holds ready filler work
behind any dependency-stalled attention matmul:
  - attention(b): per i-superblock, per causal 128-row j tile:
      S^T pair [128, 2, w] PSUM: the two heads' K=64 matmuls sit in
        disjoint PE row groups -> run CONCURRENTLY
      P^T = exp(S^T/64)  (ONE ACT instr over the 2-bank 3D AP)
      causal diag: one DVE multiply zeroes both heads' triangles
      per head: Y^T[65, 512] += V2aug_j.T @ P^T (row 64 = denominator;
        PV emission lags one j step so PE never waits on ACT)
      normalize: y = Y^T[0:64] * bcast(1/Y^T[64])
  - QKV(b+1): W.T @ x.T + bias (DVE), then V^T transposes into V2aug
  - proj(b-1): y.T @ Wp^T -> staged bf16 -> DMA out [B, T, C] bf16
Host: out = sum(partials f32) + b_proj.
"""

import os
import sys
from itertools import chain

import numpy as np

os.environ.setdefault("MYCRO_LOCAL_CACHE", "1")
if "/opt/trn_rl_repo" not in sys.path:
    sys.path.insert(0, "/opt/trn_rl_repo")

B, T, C = 4, 2048, 1024
H, D = 16, 64
N_CORES = 8
HPC = H // N_CORES          # heads per core = 2
NL = HPC * D                # local width per q/k/v = 128
KT = C // 128               # 8 contraction tiles for QKV
NT = 3                      # q, k, v
SW = 512                    # i superblock width
NSB = T // SW               # 4 superblocks per batch
NJT = T // 128              # 16 j tiles per batch
FILL = 1                    # filler units pulled per attention j-slot

# matmul input dtype: bf16 (fastest), f32r (tf32-like), f32 (exact, 4x slow)
KDT = os.environ.get("KERNEL_DTYPE", "bf16")

_cache = {}
LAST_RESULT = None


def _np_mdt():
    if KDT == "bf16":
        import ml_dtypes
        return np.dtype(ml_dtypes.bfloat16)
    return np.dtype(np.float32)


def _build():
    import concourse.tile as tile
    from concourse import bacc, mybir

    dt = mybir.dt
    f32 = dt.float32
    mdt = {"bf16": dt.bfloat16, "f32r": dt.float32r, "f32": f32}[KDT]

    nc = bacc.Bacc("TRN2", target_bir_lowering=False, debug=False,
                   num_devices=N_CORES)

    xt = nc.dram_tensor("xt", [B, C, T], mdt, kind="ExternalInput").ap()
    wqkv = nc.dram_tensor("wqkv", [128, KT, NT * 128], mdt,
                          kind="ExternalInput").ap()
    bqkv = nc.dram_tensor("bqkv", [128, NT], f32, kind="ExternalInput").ap()
    wp = nc.dram_tensor("wp", [128, C], mdt, kind="ExternalInput").ap()
    out = nc.dram_tensor("out", [B, T, C], mdt, kind="ExternalOutput").ap()

    np_m = _np_mdt() if KDT == "bf16" else np.float32
    ident_np = np.eye(128).astype(np_m)
    # P^T layout: rows x = j (keys), cols y = i (queries); keep j <= i
    # 0/1 multiplicative causal mask applied post-exp, duplicated per head
    trit01_np = np.where(
        np.arange(128)[:, None] <= np.arange(128)[None, :],
        np.float32(1.0), np.float32(0.0)).astype(np_m)
    trit01_2_np = np.ascontiguousarray(
        np.stack([trit01_np, trit01_np], axis=1))  # [128, 2, 128]
    ones_np = np.ones((128, NJT, HPC, 1)).astype(np_m)
    ident_dram = nc.inline_tensor(ident_np, name="ident").ap()
    trit2_dram = nc.inline_tensor(trit01_2_np, name="tritmask2").ap()
    ones_dram = nc.inline_tensor(ones_np, name="onescol").ap()

    Exp = mybir.ActivationFunctionType.Exp

    _alt = [0]

    def copy_alt(dst, src):
        _alt[0] ^= 1
        if _alt[0]:
            nc.scalar.copy(dst, src)
        else:
            nc.vector.tensor_copy(dst, src)

    with tile.TileContext(nc) as tc:
        with (
            tc.tile_pool(name="consts", bufs=1) as consts,
            tc.tile_pool(name="xtp", bufs=2) as xtp,
            tc.tile_pool(name="qkvtp", bufs=2) as qkvtp,
            tc.tile_pool(name="yp", bufs=2) as yp,
            tc.tile_pool(name="v2p", bufs=2) as v2p,
            tc.tile_pool(name="ptp", bufs=8) as ptp,
            tc.tile_pool(name="stage", bufs=8) as stage,
            tc.tile_pool(name="stats", bufs=6) as stats,
            tc.tile_pool(name="rbp", bufs=2) as rbp,
            tc.tile_pool(name="ycp", bufs=2) as ycp,
            tc.tile_pool(name="s_ps", bufs=2, space="PSUM") as s_ps,
            tc.tile_pool(name="yt_ps", bufs=2, space="PSUM") as yt_ps,
            tc.tile_pool(name="qk_ps", bufs=2, space="PSUM") as qk_ps,
        ):
            # HAM warm-up primer: dense dummy matmuls with no input deps so
            # the PE clock is at 2.4GHz by the time real work arrives.
            prime = consts.tile([128, SW], mdt if KDT != "f32r" else f32)
            nc.gpsimd.memset(prime[:], 0.25)
            for _ in range(0 if KDT == "f32r" else 40):
                pps = qk_ps.tile([128, SW], f32, tag="m")
                nc.tensor.matmul(pps[:], lhsT=prime[:, 0:128], rhs=prime[:],
                                 start=True, stop=True)

            nc.scalar.activation(prime[0:1, 0:1], prime[0:1, 0:1], Exp,
                                 scale=1.0)

            wqkv_sb = consts.tile([128, KT, NT * 128], mdt)
            nc.sync.dma_start(wqkv_sb[:], wqkv[:])
            wp_sb = consts.tile([128, C], mdt)
            nc.sync.dma_start(wp_sb[:], wp[:])
            bias_sb = consts.tile([128, NT], f32)
            nc.sync.dma_start(bias_sb[:], bqkv[:])
            ident_sb = consts.tile([128, 128], mdt)
            nc.sync.dma_start(ident_sb[:], ident_dram[:].bitcast(mdt))
            trit2_sb = consts.tile([128, 2, 128], mdt)
            nc.sync.dma_start(trit2_sb[:], trit2_dram[:].bitcast(mdt))

            qkvt_of = {}
            v2a_of = {}
            y_of = {}

            def qkv_stream(b):
                """QKV matmuls + bias, then V^T transposes. Yields after
                every couple of PE instructions."""
                xt_sb = xtp.tile([128, KT, T], mdt, tag="xt")
                for k in range(KT):
                    nc.sync.dma_start(xt_sb[:, k, :],
                                      xt[b, k * 128:(k + 1) * 128, :])
                qkvt = qkvt_of[b] = qkvtp.tile([128, NT, T], mdt, tag="qkvt", name="qkvt")
                for n_t in range(NT):
                    for ts in range(T // SW):
                        ps = qk_ps.tile([128, SW], f32, tag="m")
                        for k in range(KT):
                            nc.tensor.matmul(
                                ps[:],
                                lhsT=wqkv_sb[:, k, n_t * 128:(n_t + 1) * 128],
                                rhs=xt_sb[:, k, ts * SW:(ts + 1) * SW],
                                start=(k == 0), stop=(k == KT - 1),
                            )
                            if k % 2 == 1:
                                yield
                        _alt[0] ^= 1
                        if _alt[0]:
                            nc.scalar.activation(
                                qkvt[:, n_t, ts * SW:(ts + 1) * SW], ps[:],
                                mybir.ActivationFunctionType.Identity,
                                bias=bias_sb[:, n_t:n_t + 1], scale=1.0)
                        else:
                            nc.vector.tensor_scalar_add(
                                qkvt[:, n_t, ts * SW:(ts + 1) * SW], ps[:],
                                bias_sb[:, n_t:n_t + 1])
                # V2aug: V^T transposed + ones column
                v2a = v2a_of[b] = v2p.tile([128, NJT, HPC, 65], mdt,
                                           tag="v2a", name="v2a")
                nc.sync.dma_start(v2a[:, :, :, 64:65],
                                   ones_dram[:].bitcast(mdt))
                for j_t in range(NJT):
                    trp = qk_ps.tile([128, 128], mdt, tag="m")
                    nc.tensor.transpose(
                        trp[:], qkvt[:, 2, j_t * 128:(j_t + 1) * 128],
                        ident_sb[:])
                    # both head halves in one strided copy [128, 2, 64]
                    nc.vector.tensor_copy(v2a[:, j_t, :, 0:64], trp[:])
                    yield

            def attention_stream(b):
                """S/exp/PV per causal j tile; heads run as concurrent PE
                row-tile pairs; PV emission lags one j step."""
                qkvt = qkvt_of[b]
                v2a = v2a_of[b]
                y_sb = y_of[b] = yp.tile([128, T], mdt, tag="y", name="y_sb")
                q0 = qkvt[0:64, 0, :]
                k0 = qkvt[0:64, 1, :]
                q1 = qkvt[64:128, 0, :]
                k1 = qkvt[64:128, 1, :]
                def _pv(yt, njt, j_t, w, pt):
                    for h in range(HPC):
                        nc.tensor.matmul(
                            yt[h][:, SW - w:SW],
                            lhsT=v2a[:, j_t, h, :],
                            rhs=pt[:, h, :w],
                            start=(j_t == 0), stop=(j_t == njt - 1),
                        )

                def _norm(yt, i_sb):
                    # normalize: y = yc[0:64] * bcast(1/yc[64]).
                    # First ops copy both accumulators to ONE SBUF tile so
                    # the yt PSUM banks free after one instruction each AND
                    # the denominator copy / reciprocal / broadcast each run
                    # ONCE over both heads (3D APs) instead of per head.
                    # (denom row bounced to base-0: the approx-recip custom
                    # op misreads PSUM/base-64 inputs on HW)
                    yc = ycp.tile([65, HPC, SW], mdt, tag="yc", name="yc")
                    for h in range(HPC):
                        copy_alt(yc[:, h, :], yt[h][:])
                    dnr = stats.tile([1, HPC, SW], f32, tag="dnr",
                                     name="dnr")
                    nc.vector.tensor_copy(dnr[:], yc[64:65, :, :])
                    rcp = stats.tile([1, HPC, SW], f32, tag="rcp",
                                     name="rcp")
                    nc.vector.reciprocal_approx_fast(out=rcp[:], in_=dnr[:])
                    rb = rbp.tile([64, HPC, SW], f32, tag="rb", name="rb")
                    nc.gpsimd.partition_broadcast(rb[:], rcp[:])
                    for h in range(HPC):
                        nc.vector.tensor_mul(
                            y_sb[h * 64:(h + 1) * 64,
                                 i_sb * SW:(i_sb + 1) * SW],
                            yc[0:64, h, :], rb[:, h, :])

                def _proj(b, i_sb):
                    # this superblock's tokens are normalized -> project
                    for m_t in range(4 * i_sb, 4 * (i_sb + 1)):
                        for c_h in range(C // SW):
                            op = qk_ps.tile([128, SW], f32, tag="m",
                                            name="op")
                            nc.tensor.matmul(
                                op[:],
                                lhsT=y_sb[:, m_t * 128:(m_t + 1) * 128],
                                rhs=wp_sb[:, c_h * SW:(c_h + 1) * SW],
                                start=True, stop=True,
                            )
                            ost = stage.tile([128, SW], mdt, tag="ost")
                            copy_alt(ost[:], op[:])
                            nc.sync.dma_start(
                                out[b, m_t * 128:(m_t + 1) * 128,
                                    c_h * SW:(c_h + 1) * SW], ost[:])

                pending = []  # deferred PV/norm emission, lag 2 j-slots
                LAG = 6
                # reversed superblock order: the big superblock's proj
                # overlaps the small ones' attention, and each batch ends
                # on the lightest norm chain (smaller tail bubble)
                for i_sb in reversed(range(NSB)):
                    yt = [yt_ps.tile([65, SW], f32, tag="yt",
                                     name=f"yt{h}")
                          for h in range(HPC)]
                    njt = 4 * (i_sb + 1)

                    for j_t in range(njt):
                        if len(pending) > LAG:
                            pending.pop(0)()
                        yield
                        jtl = j_t - 4 * i_sb   # >=0 on the diagonal
                        diag = jtl >= 0
                        w = SW - jtl * 128 if diag else SW
                        i_lo = j_t * 128 if diag else i_sb * SW
                        sp = s_ps.tile([128, 2, SW], f32, tag="s")
                        # two K=64 matmuls in disjoint PE row groups ->
                        # concurrent (tile_position auto (0,0) / (64,0))
                        nc.tensor.matmul(
                            sp[:, 0, :w],
                            lhsT=k0[:, j_t * 128:(j_t + 1) * 128],
                            rhs=q0[:, i_lo:i_lo + w],
                            start=True, stop=True,
                        )
                        nc.tensor.matmul(
                            sp[:, 1, :w],
                            lhsT=k1[:, j_t * 128:(j_t + 1) * 128],
                            rhs=q1[:, i_lo:i_lo + w],
                            start=True, stop=True,
                        )
                        # one exp over both heads' banks (3D AP)
                        pt = ptp.tile([128, 2, SW], mdt, tag="pt")
                        nc.scalar.activation(
                            pt[:, :, :w], sp[:, :, :w], Exp, scale=1.0 / D)
                        if diag:
                            # causal mask: zero both heads' triangles with
                            # one DVE multiply (post-exp, 0/1 mask)
                            nc.vector.tensor_mul(
                                pt[:, :, 0:128], pt[:, :, 0:128],
                                trit2_sb[:])
                        pending.append(
                            lambda yt=yt, njt=njt, j_t=j_t, w=w, pt=pt:
                                _pv(yt, njt, j_t, w, pt))
                    pending.append(
                        lambda yt=yt, i_sb=i_sb: _norm(yt, i_sb))
                    pending.append(
                        lambda i_sb=i_sb: _proj(b, i_sb))
                while pending:
                    pending.pop(0)()
                    yield

            # prologue: batch 0's QKV runs alone
            for _ in qkv_stream(0):
                pass
            for b in range(B):
                filler = qkv_stream(b + 1) if b + 1 < B else iter(())
                for _ in attention_stream(b):
                    for _ in range(FILL):
                        if next(filler, None) is None:
                            break
                for _ in filler:
                    pass

    nc.compile()
    return nc


def _get_nc():
    if "nc" not in _cache:
        _cache["nc"] = _build()
    return _cache["nc"]


def kernel(x, W_attn, b_attn, W_proj, b_proj):
    global LAST_RESULT
    from concourse.bass_utils import run_bass_kernel_spmd

    x = np.asarray(x, dtype=np.float32)
    W_attn = np.asarray(W_attn, dtype=np.float32)
    b_attn = np.asarray(b_attn, dtype=np.float32)
    W_proj = np.asarray(W_proj, dtype=np.float32)
    b_proj = np.asarray(b_proj, dtype=np.float32)

    nc = _get_nc()
    np_m = _np_mdt()

    xt = np.ascontiguousarray(x.transpose(0, 2, 1)).astype(np_m)
    in_maps = []
    for c in range(N_CORES):
        sl = slice(c * NL, (c + 1) * NL)
        w_shard = np.concatenate(
            [W_attn[sl], W_attn[C:2 * C][sl], W_attn[2 * C:][sl]], axis=0)
        # wqkv[p, k, n] = w_shard.T[k*128+p, n]
        wqkv = np.ascontiguousarray(
            w_shard.T.reshape(KT, 128, NT * 128).transpose(1, 0, 2)).astype(np_m)
        b_shard = np.concatenate(
            [b_attn[sl], b_attn[C:2 * C][sl], b_attn[2 * C:][sl]])
        bq = np.ascontiguousarray(b_shard.reshape(NT, 128).T)
        wp_c = np.ascontiguousarray(W_proj[:, sl].T).astype(np_m)
        in_maps.append({"xt": xt, "wqkv": wqkv, "bqkv": bq, "wp": wp_c})

    try:
        res = run_bass_kernel_spmd(nc, in_maps,
                                   core_ids=list(range(N_CORES)))
    except Exception:
        # one retry: transient NRT/device hiccups recover on re-run
        import time
        time.sleep(10)
        res = run_bass_kernel_spmd(nc, in_maps,
                                   core_ids=list(range(N_CORES)))
    LAST_RESULT = res

    acc = res.results[0]["out"].astype(np.float32)
    for c in range(1, N_CORES):
        acc = acc + res.results[c]["out"].astype(np.float32)
    return acc + b_proj
